# Initial kernel scaffold
#
"""Detection-criterion loss kernel for Trainium2 (8 NeuronCores, SPMD).

loss = 2*class_bce + 4*xywh_sse + obj_bce   summed over 6M (batch*anchor) rows.

Math trick: for a binary target t and prob p,
    t*log(p) + (1-t)*log(1-p) = log|p + t - 1| = 0.5 * log((t_bar - p)^2)
with t_bar = 1 - t. So each BCE term needs one subtract, one square, one log
-- no select. The one-hot class target is fused into the subtract via
scalar_tensor_tensor: s_j = (cls != j) - p_j  (j = 0..2), and the weighted sum
2*class + obj is recovered at the end from separately reduced regions:
    total = 4*sum(d^2) - sum(ln(prod_j s_j^2)) - 0.5*sum(ln(s_obj^2)).

Per-core layout (25 tiles of 128 partitions x 235 rows):
    X = [ d (3R) | s_cls planar (3R) | s_obj (R) ]   (DVE, 5 ops)
    Y[0:7R] = Square(X)                              (ACT, 1 op)
    Y[7R:8R] = P2 = qc0*qc1*qc2                      (GPSIMD, 2 ops)
    LL = Ln(Y[6R:8R]) = [ln q_obj^2 | ln P2]         (ACT, 1 op)
    psum += ones.T @ [Y[0:3R], LL]                   (PE matmuls, accumulating)
Host sums the 8 per-core [1, 5R] partial vectors in float64.
"""

import numpy as np

P = 128                  # SBUF partitions
R = 235                  # rows per partition per tile
TILE_ROWS = P * R        # 30080
T = 25                   # tiles per core
CORE_ROWS = T * TILE_ROWS  # 752000
N_CORES = 8
TOTAL_ROWS = 2_000_000 * 3

_CACHE = {}


def _build_module(reps: int = 1, io_bufs: int = 3, work_bufs: int = 2):
    import concourse.bacc as bacc
    import concourse.bass as bass
    import concourse.tile as tile
    from concourse import mybir

    f32 = mybir.dt.float32
    AF = mybir.ActivationFunctionType
    OP = mybir.AluOpType

    nc = bacc.Bacc(None, target_bir_lowering=False)

    o_d = nc.dram_tensor("o", [CORE_ROWS, 7], f32, kind="ExternalInput")
    g_d = nc.dram_tensor("g", [CORE_ROWS, 5], f32, kind="ExternalInput")
    res_d = nc.dram_tensor("res", [1, 5 * R], f32, kind="ExternalOutput")

    R3 = 3 * R

    with tile.TileContext(nc) as tc:
        with (
            tc.tile_pool(name="io", bufs=io_bufs) as io,
            tc.tile_pool(name="work", bufs=work_bufs) as work,
            tc.tile_pool(name="consts", bufs=1) as consts,
            tc.tile_pool(name="ps", bufs=1, space=bass.MemorySpace.PSUM) as ps,
        ):
            ones = consts.tile([P, 1], f32)
            nc.vector.memset(ones[:], 1.0)

            psum_sq = ps.tile([1, R3], f32)      # sum of d^2 per free slot
            psum_ll = ps.tile([1, 2 * R], f32)   # [ln q_obj^2 | ln P2] sums

            for rep, t in ((rp, tt) for rp in range(reps) for tt in range(T)):
                lo = t * TILE_ROWS
                so = io.tile([P, R, 7], f32, tag="so")
                sg = io.tile([P, R, 5], f32, tag="sg")
                nc.sync.dma_start(
                    out=so[:],
                    in_=o_d[lo : lo + TILE_ROWS, :].rearrange(
                        "(p j) c -> p j c", p=P
                    ),
                )
                nc.sync.dma_start(
                    out=sg[:],
                    in_=g_d[lo : lo + TILE_ROWS, :].rearrange(
                        "(p j) c -> p j c", p=P
                    ),
                )

                x = work.tile([P, 7 * R], f32, tag="x")
                y = work.tile([P, 8 * R], f32, tag="y")
                ll = work.tile([P, 2 * R], f32, tag="ll")
                p1 = work.tile([P, R], f32, tag="p1")

                # coord diffs, planar [c-major] so Y[0:3R] is dense
                nc.vector.tensor_sub(
                    x[:, 0:R3].rearrange("p (c r) -> p r c", c=3),
                    so[:, :, 1:4],
                    sg[:, :, 1:4],
                )
                # class terms: s_j = (cls != j) - p_j
                for j in range(3):
                    nc.vector.scalar_tensor_tensor(
                        out=x[:, R3 + j * R : R3 + (j + 1) * R],
                        in0=sg[:, :, 4],
                        scalar=float(j),
                        in1=so[:, :, 4 + j],
                        op0=OP.not_equal,
                        op1=OP.subtract,
                    )
                # obj term: s = (obj == 0) - p_obj  (= (1-t) - p)
                nc.vector.scalar_tensor_tensor(
                    out=x[:, 6 * R : 7 * R],
                    in0=sg[:, :, 0],
                    scalar=0.0,
                    in1=so[:, :, 0],
                    op0=OP.is_equal,
                    op1=OP.subtract,
                )
                # squares: Y[0:3R]=d^2, [3R:6R]=qc2, [6R:7R]=q_obj^2
                nc.scalar.activation(y[:, 0 : 7 * R], x[:, 0 : 7 * R], AF.Square)
                # class product P2 = qc2_0 * qc2_1 * qc2_2 -> Y[7R:8R]
                nc.gpsimd.tensor_mul(p1[:], y[:, R3 : 4 * R], y[:, 4 * R : 5 * R])
                nc.gpsimd.tensor_mul(y[:, 7 * R : 8 * R], p1[:], y[:, 5 * R : 6 * R])
                # logs over contiguous [q_obj^2 | P2]
                nc.scalar.activation(ll[:], y[:, 6 * R : 8 * R], AF.Ln)

                # reductions over partitions, accumulated over tiles in PSUM
                st = (t == 0) and (rep == 0)
                sp = (t == T - 1) and (rep == reps - 1)
                nc.tensor.matmul(
                    psum_sq[:, 0:512], ones[:], y[:, 0:512], start=st, stop=sp
                )
                nc.tensor.matmul(
                    psum_sq[:, 512:R3], ones[:], y[:, 512:R3], start=st, stop=sp
                )
                nc.tensor.matmul(
                    psum_ll[:], ones[:], ll[:], start=st, stop=sp
                )

            out_sb = consts.tile([1, 5 * R], f32)
            nc.vector.tensor_copy(out_sb[:, 0:R3], psum_sq[:])
            nc.vector.tensor_copy(out_sb[:, R3 : 5 * R], psum_ll[:])
            nc.sync.dma_start(res_d[:], out_sb[:])

    nc.compile()
    return nc


def _get_module(reps: int = 1, io_bufs: int = 3, work_bufs: int = 2):
    key = ("nc", reps, io_bufs, work_bufs)
    if key not in _CACHE:
        _CACHE[key] = _build_module(reps, io_bufs, work_bufs)
    return _CACHE[key]


def kernel(output: np.ndarray, target: np.ndarray) -> np.ndarray:
    from concourse.bass_utils import run_bass_kernel_spmd

    o = np.ascontiguousarray(output, dtype=np.float32).reshape(TOTAL_ROWS, 7)
    g = np.ascontiguousarray(target, dtype=np.float32).reshape(TOTAL_ROWS, 5)

    in_maps = []
    for c in range(N_CORES):
        lo = c * CORE_ROWS
        hi = min(lo + CORE_ROWS, TOTAL_ROWS)
        oc, gc = o[lo:hi], g[lo:hi]
        if hi - lo < CORE_ROWS:
            padn = CORE_ROWS - (hi - lo)
            opad = np.zeros((padn, 7), np.float32)
            gpad = np.zeros((padn, 5), np.float32)
            gpad[:, 4] = -1.0  # class id outside [0,3) -> zero loss contribution
            oc = np.concatenate([oc, opad])
            gc = np.concatenate([gc, gpad])
        in_maps.append({"o": oc, "g": gc})

    nc = _get_module()
    r = run_bass_kernel_spmd(nc, in_maps, core_ids=list(range(N_CORES)))

    R3 = 3 * R
    total = 0.0
    for c in range(N_CORES):
        res = np.asarray(r.results[c]["res"]).reshape(-1).astype(np.float64)
        total += (
            4.0 * res[0:R3].sum()
            - 0.5 * res[R3 : 4 * R].sum()
            - res[4 * R : 5 * R].sum()
        )
    return np.array(total, dtype=np.float32)



# revision 2
# speedup vs baseline: 1.0515x; 1.0515x over previous
"""Detection-criterion loss kernel for Trainium2 (8 NeuronCores, SPMD) — v2.

loss = 2*class_bce + 4*xywh_sse + obj_bce   summed over 6M (batch*anchor) rows.

Math trick: for binary target t and prob p,
    t*log(p) + (1-t)*log(1-p) = log|p + t - 1| = 0.5 * log((t_bar - p)^2)
so each BCE term is one subtract + square + log:
    total = 4*sum(d^2) - sum(ln q_cls^2) - 0.5*sum(ln q_obj^2)

v2 vs baseline:
  * host converts inputs to bf16 (loss tolerance is 2e-2; rounding noise
    cancels in 24M-term sums) -> HBM traffic halved
  * host repacks to a planar, tile-shaped layout [T, P, 12 planes, R] so
    every device-side DVE op is contiguous step-1 bf16 (2x perf mode)
    and each DMA is one big linear transfer
  * ACT only runs Ln, with fused accum_out (per-partition sums) so the
    ln regions never hit the PE; squares split DVE (coords) / GPSIMD (q)
  * PE reduces only d^2, in bf16 (1 col/cycle instead of fp32's ~3.5)

Plane order in the packed buffer (12 planes of R rows):
  0-2: o coords | 3-5: g coords | 6: p_obj | 7-9: p_cls | 10: t_obj | 11: cls_idx
DMA1 brings planes 0-5 (feeds sub/d^2), DMA2 planes 6-11 (feeds the q's).

o is clamped to <= 1-2^-9 before bf16 so no prob rounds to exactly 1.0
(q would be 0 -> ln = -inf). Pad rows use cls=-1, everything else 0,
which contributes exactly zero loss.
"""

import numpy as np

P = 128                    # SBUF partitions
R = 1184                   # rows per partition per tile (even: keeps 2x DVE mode)
T = 5                      # tiles per core
TILE_ROWS = P * R          # 151552
CORE_ROWS = T * TILE_ROWS  # 757760
N_CORES = 8
TOTAL_ROWS = 2_000_000 * 3
TOTAL_PADDED = N_CORES * CORE_ROWS  # 6062080

CLAMP = 1.0 - 2.0 ** -8    # largest bf16 strictly below 1.0 (7 mantissa bits)

# combined column -> plane permutation ([o cols 0..6 | g cols 0..4])
PERM = [1, 2, 3, 8, 9, 10, 0, 4, 5, 6, 7, 11]

NRES = 512 + 2 * T         # d2 psum (512) | cls sums (T) | obj sums (T)

_CACHE = {}


def _build_module(io_bufs: int = 3, work_bufs: int = 2, gpsimd_stt: int = 0,
                  use_prod: bool = False):
    import concourse.bacc as bacc
    import concourse.bass as bass
    import concourse.tile as tile
    from concourse import mybir

    f32 = mybir.dt.float32
    bf16 = mybir.dt.bfloat16
    AF = mybir.ActivationFunctionType
    OP = mybir.AluOpType

    nc = bacc.Bacc(None, target_bir_lowering=False)

    x_d = nc.dram_tensor("x", [T, P, 12 * R], bf16, kind="ExternalInput")
    res_d = nc.dram_tensor("res", [1, NRES], f32, kind="ExternalOutput")

    R3, R4, R6, R7 = 3 * R, 4 * R, 6 * R, 7 * R

    with tile.TileContext(nc) as tc:
        with (
            tc.tile_pool(name="io", bufs=io_bufs) as io,
            tc.tile_pool(name="work", bufs=work_bufs) as work,
            tc.tile_pool(name="consts", bufs=1) as consts,
            tc.tile_pool(name="ps", bufs=1, space=bass.MemorySpace.PSUM) as ps,
        ):
            ones_bf = consts.tile([P, 1], bf16)
            nc.vector.memset(ones_bf[:], 1.0)
            ones_f32 = consts.tile([P, 1], f32)
            nc.vector.memset(ones_f32[:], 1.0)
            acc = consts.tile([P, 2 * T], f32)  # [cls sums (T) | obj sums (T)]

            psum_d2 = ps.tile([1, 512], f32)
            psum_fin = ps.tile([1, 2 * T], f32)

            # d^2 chunk boundaries within [0, 3R)
            cuts = list(range(0, R3, 512)) + [R3]
            n_chunks = len(cuts) - 1

            for t in range(T):
                ioA = io.tile([P, R6], bf16, tag="ioA")  # coords (o then g)
                ioB = io.tile([P, R6], bf16, tag="ioB")  # probs + targets
                nc.sync.dma_start(out=ioA[:], in_=x_d[t, :, 0:R6])
                nc.sync.dma_start(out=ioB[:], in_=x_d[t, :, R6 : 12 * R])

                w = work.tile([P, R7], bf16, tag="w")  # [d 3R | q0..q2 | q_obj]
                y = work.tile([P, R7], bf16, tag="y")  # squares of w
                ll = work.tile([P, R4], bf16, tag="ll")  # Ln outputs (discarded)

                # d = o_coord - t_coord (one 3R-wide op, all contiguous)
                nc.vector.tensor_sub(w[:, 0:R3], ioA[:, 0:R3], ioA[:, R3:R6])
                # q_j = (cls != j) - p_cls_j   (optionally some on gpsimd)
                for j in range(3):
                    eng = nc.gpsimd if j < gpsimd_stt else nc.vector
                    eng.scalar_tensor_tensor(
                        out=w[:, (3 + j) * R : (4 + j) * R],
                        in0=ioB[:, 5 * R : R6],
                        scalar=float(j),
                        in1=ioB[:, (1 + j) * R : (2 + j) * R],
                        op0=OP.not_equal,
                        op1=OP.subtract,
                    )
                # q_obj = (t_obj == 0) - p_obj
                eng = nc.gpsimd if gpsimd_stt > 3 else nc.vector
                eng.scalar_tensor_tensor(
                    out=w[:, R6:R7],
                    in0=ioB[:, R4 : 5 * R],
                    scalar=0.0,
                    in1=ioB[:, 0:R],
                    op0=OP.is_equal,
                    op1=OP.subtract,
                )

                # d^2 on DVE (distinct-port trick not needed; 2x holds)
                nc.vector.tensor_mul(y[:, 0:R3], w[:, 0:R3], w[:, 0:R3])

                if use_prod:
                    # prod = q0*q1*q2; ln(prod^2) = sum_j ln(q_j^2)
                    nc.vector.tensor_mul(y[:, R3:R4], w[:, R3:R4], w[:, R4:5 * R])
                    nc.vector.tensor_mul(y[:, R4:5 * R], y[:, R3:R4], w[:, 5 * R:R6])
                    nc.scalar.activation(y[:, 5 * R:R6], y[:, R4:5 * R], AF.Square)
                    nc.scalar.activation(y[:, R6:R7], w[:, R6:R7], AF.Square)
                    nc.scalar.activation(
                        ll[:, 0:R], y[:, 5 * R:R6], AF.Ln,
                        accum_out=acc[:, t : t + 1],
                    )
                    nc.scalar.activation(
                        ll[:, R:2 * R], y[:, R6:R7], AF.Ln,
                        accum_out=acc[:, T + t : T + t + 1],
                    )
                else:
                    # ACT squares the q planes, then Ln with fused accums
                    nc.scalar.activation(y[:, R3:R7], w[:, R3:R7], AF.Square)
                    nc.scalar.activation(
                        ll[:, 0:R3], y[:, R3:R6], AF.Ln,
                        accum_out=acc[:, t : t + 1],
                    )
                    nc.scalar.activation(
                        ll[:, R3:R4], y[:, R6:R7], AF.Ln,
                        accum_out=acc[:, T + t : T + t + 1],
                    )

                # PE: column-sum the d^2 region into psum (bf16 moving)
                for ci in range(n_chunks):
                    lo, hi = cuts[ci], cuts[ci + 1]
                    nc.tensor.matmul(
                        psum_d2[:, 0 : hi - lo],
                        ones_bf[:],
                        y[:, lo:hi],
                        start=(t == 0 and ci == 0),
                        stop=(t == T - 1 and ci == n_chunks - 1),
                    )

            # partition-reduce the ACT accumulators
            nc.tensor.matmul(
                psum_fin[:], ones_f32[:], acc[:], start=True, stop=True
            )

            out_sb = consts.tile([1, NRES], f32)
            nc.vector.tensor_copy(out_sb[:, 0:512], psum_d2[:])
            nc.vector.tensor_copy(out_sb[:, 512:NRES], psum_fin[:])
            nc.sync.dma_start(res_d[:], out_sb[:])

    nc.compile()
    return nc


def _get_module(**kw):
    key = tuple(sorted(kw.items()))
    if key not in _CACHE:
        _CACHE[key] = _build_module(**kw)
    return _CACHE[key]


def _pack_inputs(output: np.ndarray, target: np.ndarray) -> np.ndarray:
    """fp32 [B,A,7]/[B,A,5] -> bf16 planar [N_CORES, T, P, 12, R]."""
    import ml_dtypes

    bf16 = np.dtype(ml_dtypes.bfloat16)
    o = np.ascontiguousarray(output, dtype=np.float32).reshape(TOTAL_ROWS, 7)
    g = np.ascontiguousarray(target, dtype=np.float32).reshape(TOTAL_ROWS, 5)

    big = np.zeros((TOTAL_PADDED, 12), dtype=bf16)
    for k, c in enumerate(PERM):
        if c < 7:
            # o columns: clamp below 1.0 so no bf16 prob rounds to 1.0
            big[:TOTAL_ROWS, k] = np.minimum(o[:, c], np.float32(CLAMP)).astype(bf16)
        else:
            big[:TOTAL_ROWS, k] = g[:, c - 7].astype(bf16)
    big[TOTAL_ROWS:, 11] = -1.0  # pad rows: class id -1 -> zero loss

    x = big.reshape(N_CORES, T, P, R, 12).transpose(0, 1, 2, 4, 3)
    return np.ascontiguousarray(x)


def kernel(output: np.ndarray, target: np.ndarray) -> np.ndarray:
    from concourse.bass_utils import run_bass_kernel_spmd

    x = _pack_inputs(output, target)
    in_maps = [{"x": x[c].reshape(T, P, 12 * R)} for c in range(N_CORES)]

    nc = _get_module()
    r = run_bass_kernel_spmd(nc, in_maps, core_ids=list(range(N_CORES)))

    total = 0.0
    for c in range(N_CORES):
        res = np.asarray(r.results[c]["res"]).reshape(-1).astype(np.float64)
        total += (
            4.0 * res[0:512].sum()
            - res[512 : 512 + T].sum()
            - 0.5 * res[512 + T : 512 + 2 * T].sum()
        )
    return np.array(total, dtype=np.float32)


# revision 3
# speedup vs baseline: 1.1535x; 1.0970x over previous
"""Detection-criterion loss kernel for Trainium2 (8 NeuronCores, SPMD) — v2.

loss = 2*class_bce + 4*xywh_sse + obj_bce   summed over 6M (batch*anchor) rows.

Math trick: for binary target t and prob p,
    t*log(p) + (1-t)*log(1-p) = log|p + t - 1| = 0.5 * log((t_bar - p)^2)
so each BCE term is one subtract + square + log:
    total = 4*sum(d^2) - sum(ln q_cls^2) - 0.5*sum(ln q_obj^2)

v2 vs baseline:
  * host converts inputs to bf16 (loss tolerance is 2e-2; rounding noise
    cancels in 24M-term sums) -> HBM traffic halved
  * host repacks to a planar, tile-shaped layout [T, P, 12 planes, R] so
    every device-side DVE op is contiguous step-1 bf16 (2x perf mode)
    and each DMA is one big linear transfer
  * ACT only runs Ln, with fused accum_out (per-partition sums) so the
    ln regions never hit the PE; squares split DVE (coords) / GPSIMD (q)
  * PE reduces only d^2, in bf16 (1 col/cycle instead of fp32's ~3.5)

Plane order in the packed buffer (12 planes of R rows):
  0-2: o coords | 3-5: g coords | 6: p_obj | 7-9: p_cls | 10: t_obj | 11: cls_idx
DMA1 brings planes 0-5 (feeds sub/d^2), DMA2 planes 6-11 (feeds the q's).

o is clamped to <= 1-2^-9 before bf16 so no prob rounds to exactly 1.0
(q would be 0 -> ln = -inf). Pad rows use cls=-1, everything else 0,
which contributes exactly zero loss.
"""

import numpy as np

P = 128                    # SBUF partitions
R = 1184                   # rows per partition per tile (even: keeps 2x DVE mode)
T = 5                      # tiles per core
TILE_ROWS = P * R          # 151552
CORE_ROWS = T * TILE_ROWS  # 757760
N_CORES = 8
TOTAL_ROWS = 2_000_000 * 3
TOTAL_PADDED = N_CORES * CORE_ROWS  # 6062080

CLAMP = 1.0 - 2.0 ** -8    # largest bf16 strictly below 1.0 (7 mantissa bits)

# combined column -> plane permutation ([o cols 0..6 | g cols 0..4])
PERM = [1, 2, 3, 8, 9, 10, 0, 4, 5, 6, 7, 11]

NRES = 3 * T               # cls sums (T) | obj sums (T) | d2 sums (T)

_CACHE = {}


def _build_module(io_bufs: int = 3, work_bufs: int = 2):
    import concourse.bacc as bacc
    import concourse.bass as bass
    import concourse.tile as tile
    from concourse import mybir

    f32 = mybir.dt.float32
    bf16 = mybir.dt.bfloat16
    AF = mybir.ActivationFunctionType
    OP = mybir.AluOpType

    nc = bacc.Bacc(None, target_bir_lowering=False)

    x_d = nc.dram_tensor("x", [T, P, 12 * R], bf16, kind="ExternalInput")
    res_d = nc.dram_tensor("res", [1, NRES], f32, kind="ExternalOutput")

    R3, R4, R6, R7 = 3 * R, 4 * R, 6 * R, 7 * R

    with tile.TileContext(nc) as tc:
        with (
            tc.tile_pool(name="io", bufs=io_bufs) as io,
            tc.tile_pool(name="work", bufs=work_bufs) as work,
            tc.tile_pool(name="consts", bufs=1) as consts,
            tc.tile_pool(name="ps", bufs=1, space=bass.MemorySpace.PSUM) as ps,
        ):
            ones_f32 = consts.tile([P, 1], f32)
            nc.vector.memset(ones_f32[:], 1.0)
            acc = consts.tile([P, 3 * T], f32)  # [cls (T) | obj (T) | d2 (T)]

            psum_fin = ps.tile([1, 3 * T], f32)

            for t in range(T):
                ioA = io.tile([P, R6], bf16, tag="ioA")  # coords (o then g)
                ioB = io.tile([P, R6], bf16, tag="ioB")  # probs + targets
                nc.sync.dma_start(out=ioA[:], in_=x_d[t, :, 0:R6])
                nc.sync.dma_start(out=ioB[:], in_=x_d[t, :, R6 : 12 * R])

                w = work.tile([P, R7], bf16, tag="w")  # [d 3R | q0..q2 | q_obj]
                y = work.tile([P, R7], bf16, tag="y")  # squares of w
                ll = work.tile([P, R4], bf16, tag="ll")  # Ln outputs (discarded)

                # d = o_coord - t_coord (one 3R-wide op, all contiguous)
                nc.vector.tensor_sub(w[:, 0:R3], ioA[:, 0:R3], ioA[:, R3:R6])
                # q_j = (cls != j) - p_cls_j
                for j in range(3):
                    nc.vector.scalar_tensor_tensor(
                        out=w[:, (3 + j) * R : (4 + j) * R],
                        in0=ioB[:, 5 * R : R6],
                        scalar=float(j),
                        in1=ioB[:, (1 + j) * R : (2 + j) * R],
                        op0=OP.not_equal,
                        op1=OP.subtract,
                    )
                # q_obj = (t_obj == 0) - p_obj
                nc.vector.scalar_tensor_tensor(
                    out=w[:, R6:R7],
                    in0=ioB[:, R4 : 5 * R],
                    scalar=0.0,
                    in1=ioB[:, 0:R],
                    op0=OP.is_equal,
                    op1=OP.subtract,
                )

                # Sum d^2 straight out of ACT's accumulator (no PE pass)
                nc.scalar.activation(
                    y[:, 0:R3], w[:, 0:R3], AF.Square,
                    accum_out=acc[:, 2 * T + t : 2 * T + t + 1],
                )
                # prod = q0*q1*q2 on DVE; ln(prod^2) = sum_j ln(q_j^2)
                nc.vector.tensor_mul(y[:, R3:R4], w[:, R3:R4], w[:, R4:5 * R])
                nc.vector.tensor_mul(y[:, R4:5 * R], y[:, R3:R4], w[:, 5 * R:R6])
                nc.scalar.activation(y[:, 5 * R:R6], y[:, R4:5 * R], AF.Square)
                nc.scalar.activation(y[:, R6:R7], w[:, R6:R7], AF.Square)
                nc.scalar.activation(
                    ll[:, 0:R], y[:, 5 * R:R6], AF.Ln,
                    accum_out=acc[:, t : t + 1],
                )
                nc.scalar.activation(
                    ll[:, R:2 * R], y[:, R6:R7], AF.Ln,
                    accum_out=acc[:, T + t : T + t + 1],
                )

            # partition-reduce the ACT accumulators
            nc.tensor.matmul(
                psum_fin[:], ones_f32[:], acc[:], start=True, stop=True
            )

            out_sb = consts.tile([1, NRES], f32)
            nc.vector.tensor_copy(out_sb[:], psum_fin[:])
            nc.sync.dma_start(res_d[:], out_sb[:])

    nc.compile()
    return nc


def _get_module(**kw):
    key = tuple(sorted(kw.items()))
    if key not in _CACHE:
        _CACHE[key] = _build_module(**kw)
    return _CACHE[key]


def _pack_inputs(output: np.ndarray, target: np.ndarray) -> np.ndarray:
    """fp32 [B,A,7]/[B,A,5] -> bf16 planar [N_CORES, T, P, 12, R]."""
    import ml_dtypes

    bf16 = np.dtype(ml_dtypes.bfloat16)
    o = np.ascontiguousarray(output, dtype=np.float32).reshape(TOTAL_ROWS, 7)
    g = np.ascontiguousarray(target, dtype=np.float32).reshape(TOTAL_ROWS, 5)

    big = np.zeros((TOTAL_PADDED, 12), dtype=bf16)
    for k, c in enumerate(PERM):
        if c < 7:
            # o columns: clamp below 1.0 so no bf16 prob rounds to 1.0
            big[:TOTAL_ROWS, k] = np.minimum(o[:, c], np.float32(CLAMP)).astype(bf16)
        else:
            big[:TOTAL_ROWS, k] = g[:, c - 7].astype(bf16)
    big[TOTAL_ROWS:, 11] = -1.0  # pad rows: class id -1 -> zero loss

    x = big.reshape(N_CORES, T, P, R, 12).transpose(0, 1, 2, 4, 3)
    return np.ascontiguousarray(x)


def kernel(output: np.ndarray, target: np.ndarray) -> np.ndarray:
    from concourse.bass_utils import run_bass_kernel_spmd

    x = _pack_inputs(output, target)
    in_maps = [{"x": x[c].reshape(T, P, 12 * R)} for c in range(N_CORES)]

    nc = _get_module()
    r = run_bass_kernel_spmd(nc, in_maps, core_ids=list(range(N_CORES)))

    total = 0.0
    for c in range(N_CORES):
        res = np.asarray(r.results[c]["res"]).reshape(-1).astype(np.float64)
        total += (
            4.0 * res[2 * T : 3 * T].sum()
            - res[0:T].sum()
            - 0.5 * res[T : 2 * T].sum()
        )
    return np.array(total, dtype=np.float32)
